# revision 8
# baseline (speedup 1.0000x reference)
"""Cosine multihead attention on 8 Trainium2 NeuronCores.

Sharding: batch*heads across cores. Core c handles batch b = c // 4 and the
4 heads [4*(c%4), 4*(c%4)+4). Each core computes its heads' q/k/v projections
(tensor-parallel slices of in_proj), full attention for its (B,H) slice, and a
partial out-projection (rank-256 contribution). The host sums the 4 partials
per batch and adds out_proj_bias.

Device layout notes:
- q,k are projected directly in transposed orientation (head_dim on
  partitions, seq on free) so QK^T needs no on-chip transpose; v is projected
  in natural orientation so it is directly the PV stationary operand.
- QK^T runs 2 heads concurrently via PE row tiling (K=64 at partition bases
  0 and 64).
- Softmax denominators come free from a ones-column appended to v (M=65 PV).
- All bf16 matmuls with fp32 PSUM accumulation; softmax/normalization math in
  fp32.
"""

import sys

if "/opt/trn_rl_repo" not in sys.path:
    sys.path.insert(0, "/opt/trn_rl_repo")

import numpy as np
import ml_dtypes

import concourse.bass as bass
import concourse.tile as tile
from concourse import bacc, mybir
from concourse.bass_utils import run_bass_kernel_spmd

S, B, E, H = 2048, 2, 1024, 16
HD = E // H            # 64
HPC = 4                # heads per core
NCORES = 8
TAU_MIN = 0.01

BF16 = ml_dtypes.bfloat16
DT_BF = mybir.dt.bfloat16
DT_F32 = mybir.dt.float32

KC_E = E // 128        # 8 contraction chunks for projections
MQ = S // 128          # 16 seq chunks of 128
NPAIR = HPC // 2       # 2 head pairs per core


def build_program():
    """Build the SPMD per-core Bass program. Same program on all 8 cores;
    all per-core differences live in the input data.

    Emission order is the schedule: mc0 projections + norms, v projection,
    then pair-0 attention with mc1 projection matmuls interleaved as PE
    fillers (keeps the PE dense so HAM stays warm), an mc1 norm burst (all
    ACT sqrts batched to avoid table thrash with Exp), then pair-1 attention
    with out-projection units interleaved, draining the tail."""
    nc = bacc.Bacc(None)

    xq = nc.dram_tensor("xq_t", [E, S], DT_BF, kind="ExternalInput")
    xk = nc.dram_tensor("xk_t", [E, S], DT_BF, kind="ExternalInput")
    xv = nc.dram_tensor("xv_t", [E, S], DT_BF, kind="ExternalInput")
    wq = nc.dram_tensor("wq_t", [E, 256], DT_BF, kind="ExternalInput")
    wk = nc.dram_tensor("wk_t", [E, 256], DT_BF, kind="ExternalInput")
    wv = nc.dram_tensor("wv_t", [E, 256], DT_BF, kind="ExternalInput")
    bq = nc.dram_tensor("b_q", [1, 256], DT_BF, kind="ExternalInput")
    bk = nc.dram_tensor("b_k", [1, 256], DT_BF, kind="ExternalInput")
    bv = nc.dram_tensor("b_v", [1, 256], DT_BF, kind="ExternalInput")
    wo = nc.dram_tensor("wo_t", [256, E], DT_BF, kind="ExternalInput")
    selk_in = nc.dram_tensor("selk", [2, 256], DT_F32, kind="ExternalInput")
    selq_in = nc.dram_tensor("selq", [2, 128], DT_F32, kind="ExternalInput")
    outp = nc.dram_tensor("out_p", [S, E], DT_F32, kind="ExternalOutput")

    with tile.TileContext(nc) as tc:
        with (
            tc.tile_pool(name="consts", bufs=1) as consts,
            tc.tile_pool(name="xin", bufs=1) as xin,
            tc.tile_pool(name="wts", bufs=1) as wts,
            tc.tile_pool(name="qk", bufs=1) as qkpool,
            tc.tile_pool(name="vsb", bufs=1) as vpool,
            tc.tile_pool(name="work", bufs=2) as work,
            tc.tile_pool(name="sqp", bufs=8) as sqp,
            tc.tile_pool(name="outs", bufs=3) as outs,
            tc.tile_pool(name="ps_mm", bufs=2, space="PSUM") as ps_mm,
            tc.tile_pool(name="ps_acc", bufs=3, space="PSUM") as ps_acc,
            tc.tile_pool(name="ps_aux", bufs=1, space="PSUM") as ps_aux,
        ):
            # ---- constants -------------------------------------------------
            ones_row = consts.tile([1, 512], DT_BF, tag="ones_row")
            nc.vector.memset(ones_row, 1.0)
            ones_hi = consts.tile([128, 64], DT_F32, tag="ones_hi")
            nc.vector.memset(ones_hi, 1.0)
            selq = consts.tile([2, 128], DT_F32, tag="selq")
            nc.sync.dma_start(out=selq, in_=selq_in[:, :])
            hsel = consts.tile([128, 2], DT_BF, tag="hsel")
            nc.vector.memset(hsel, 0.0)
            nc.vector.memset(hsel[0:64, 0:1], 1.0)
            nc.vector.memset(hsel[64:128, 1:2], 1.0)
            selk_sb = consts.tile([2, 256], DT_F32, tag="selk")
            nc.sync.dma_start(out=selk_sb, in_=selk_in[:, :])

            bq_sb = consts.tile([1, 256], DT_BF, tag="bq")
            bk_sb = consts.tile([1, 256], DT_BF, tag="bk")
            bv_sb = consts.tile([1, 256], DT_BF, tag="bv")
            nc.sync.dma_start(out=bq_sb, in_=bq[:, :])
            nc.sync.dma_start(out=bk_sb, in_=bk[:, :])
            nc.sync.dma_start(out=bv_sb, in_=bv[:, :])

            # ---- weights ---------------------------------------------------
            wq_sb = wts.tile([128, KC_E, 256], DT_BF, tag="wq")
            wk_sb = wts.tile([128, KC_E, 256], DT_BF, tag="wk")
            wv_sb = wts.tile([128, KC_E, 256], DT_BF, tag="wv")
            for c in range(KC_E):
                nc.sync.dma_start(out=wq_sb[:, c, :], in_=wq[c * 128:(c + 1) * 128, :])
                nc.gpsimd.dma_start(out=wk_sb[:, c, :], in_=wk[c * 128:(c + 1) * 128, :])
                nc.sync.dma_start(out=wv_sb[:, c, :], in_=wv[c * 128:(c + 1) * 128, :])
            wo_sb = wts.tile([128, 2, E], DT_BF, tag="wo")
            for c in range(2):
                nc.sync.dma_start(out=wo_sb[:, c, :], in_=wo[c * 128:(c + 1) * 128, :])

            # ---- activations (fully resident; xq/xv on HWDGE, xk on SWDGE
            # so the two streams run on different queues) --------------------
            xq_sb = xin.tile([128, KC_E, S], DT_BF, tag="xq")
            xk_sb = xin.tile([128, KC_E, S], DT_BF, tag="xk")
            xv_sb = xin.tile([128, KC_E, S], DT_BF, tag="xv")
            for c in range(KC_E):
                nc.sync.dma_start(out=xq_sb[:, c, :], in_=xq[c * 128:(c + 1) * 128, :])
                nc.gpsimd.dma_start(out=xk_sb[:, c, :], in_=xk[c * 128:(c + 1) * 128, :])
            for c in range(KC_E):
                nc.sync.dma_start(out=xv_sb[:, c, :], in_=xv[c * 128:(c + 1) * 128, :])

            qt = [qkpool.tile([128, S], DT_BF, tag=f"qt{p}", name=f"qt{p}")
                  for p in range(NPAIR)]
            kt = [qkpool.tile([128, S], DT_BF, tag=f"kt{p}", name=f"kt{p}")
                  for p in range(NPAIR)]
            heads_t = [qkpool.tile([128, S], DT_BF, tag=f"ht{p}", name=f"ht{p}")
                       for p in range(NPAIR)]

            def proj_unit_ops(dst, w_sb, b_sb, x_sb, mc, n4):
                """Closures for one 512-wide projection unit (8 accum matmuls
                + bias matmul + psum drain/square). Returns (ops, sq_tile_box)."""
                sl = slice(n4 * 512, (n4 + 1) * 512)
                st8 = {}
                ops = []

                def mk_mm(c):
                    def go():
                        if c == 0:
                            st8["pp"] = ps_aux.tile([128, 512], DT_F32, tag="aux", name="pp_aux")
                        nc.tensor.matmul(
                            st8["pp"],
                            lhsT=w_sb[:, c, mc * 128:(mc + 1) * 128],
                            rhs=x_sb[:, c, sl],
                            start=(c == 0),
                            stop=False,
                        )
                    return go

                for c in range(KC_E):
                    ops.append(mk_mm(c))

                def bias_mm():
                    nc.tensor.matmul(
                        st8["pp"],
                        lhsT=b_sb[0:1, mc * 128:(mc + 1) * 128],
                        rhs=ones_row[0:1, 0:512],
                        start=False,
                        stop=True,
                    )
                ops.append(bias_mm)

                def drain():
                    nc.vector.tensor_copy(out=dst[:, sl], in_=st8["pp"])
                    sq = sqp.tile([128, 512], DT_BF, tag="sq", name="sq_t")
                    nc.vector.tensor_mul(sq, dst[:, sl], dst[:, sl])
                    st8["sq"] = sq
                ops.append(drain)
                return ops, st8

            def norm_unit(dst, sel, sq, n4):
                """sumsq -> sqrt -> selector-broadcast (tau folded) ->
                fast reciprocal -> in-place normalize of a 512 block."""
                sl = slice(n4 * 512, (n4 + 1) * 512)
                ss = ps_acc.tile([2, 512], DT_F32, tag="oacc", name="ss_t")
                nc.tensor.matmul(ss, lhsT=hsel, rhs=sq, start=True, stop=True)
                st = work.tile([2, 512], DT_F32, tag="st", name="st_t")
                nc.scalar.activation(st, ss, mybir.ActivationFunctionType.Sqrt)
                rb = ps_aux.tile([128, 512], DT_F32, tag="aux", name="rb_t")
                nc.tensor.matmul(rb, lhsT=sel, rhs=st, start=True, stop=True)
                rbi = work.tile([128, 512], DT_F32, tag="rbi", name="rbi_t")
                nc.vector.reciprocal_approx_fast(out=rbi, in_=rb)
                nc.vector.tensor_mul(dst[:, sl], dst[:, sl], rbi)

            # ---- phase 1: mc0 projections + norms --------------------------
            for dst, w_sb, b_sb, x_sb, sel in (
                (qt[0], wq_sb, bq_sb, xq_sb, selq),
                (kt[0], wk_sb, bk_sb, xk_sb, selk_sb[:, 0:128]),
            ):
                for n4 in range(4):
                    ops, st8 = proj_unit_ops(dst, w_sb, b_sb, x_sb, 0, n4)
                    for op in ops:
                        op()
                    norm_unit(dst, sel, st8["sq"], n4)

            # ---- phase 2: v projection (natural orientation) ---------------
            v_sb = vpool.tile([128, MQ, HPC, HD + 1], DT_BF, tag="v")
            nc.vector.memset(v_sb[:, :, :, HD:HD + 1], 1.0)
            for m in range(MQ):
                vp = ps_acc.tile([128, 256], DT_F32, tag="oacc", name="vp_t")
                for c in range(KC_E):
                    nc.tensor.matmul(
                        vp,
                        lhsT=xv_sb[:, c, m * 128:(m + 1) * 128],
                        rhs=wv_sb[:, c, :],
                        start=(c == 0),
                        stop=False,
                    )
                nc.tensor.matmul(
                    vp,
                    lhsT=ones_row[0:1, 0:128],
                    rhs=bv_sb[0:1, :],
                    start=False,
                    stop=True,
                )
                nc.vector.tensor_copy(
                    out=v_sb[:, m, :, 0:HD],
                    in_=vp.rearrange("p (h d) -> p h d", h=HPC),
                )

            # mc1 projection fillers, interleaved into pair-0 attention
            fillers = []
            mc1_sq = {}
            for key, dst, w_sb, b_sb, x_sb in (
                ("q", qt[1], wq_sb, bq_sb, xq_sb),
                ("k", kt[1], wk_sb, bk_sb, xk_sb),
            ):
                for n4 in range(4):
                    ops, st8 = proj_unit_ops(dst, w_sb, b_sb, x_sb, 1, n4)
                    fillers.extend(ops)
                    mc1_sq[key, n4] = st8

            def attention_pair(p, filler_queue, per_iter, after_qb=None):
                for qb in range(4):
                    sl_q = slice(qb * 512, (qb + 1) * 512)
                    o0 = ps_acc.tile([128, 512], DT_F32, tag="oacc", name="o0_t")
                    o1 = ps_acc.tile([128, 512], DT_F32, tag="oacc", name="o1_t")
                    for kc in range(MQ):
                        sc = ps_mm.tile([128, 1024], DT_F32, tag="sc", name="sc_t")
                        nc.tensor.matmul(
                            sc[:, 0:512],
                            lhsT=kt[p][0:64, kc * 128:(kc + 1) * 128],
                            rhs=qt[p][0:64, sl_q],
                            start=True, stop=True,
                        )
                        nc.tensor.matmul(
                            sc[:, 512:1024],
                            lhsT=kt[p][64:128, kc * 128:(kc + 1) * 128],
                            rhs=qt[p][64:128, sl_q],
                            start=True, stop=True,
                        )
                        ex = work.tile([128, 1024], DT_BF, tag="exp", name="ex_t")
                        nc.scalar.activation(
                            ex, sc, mybir.ActivationFunctionType.Exp
                        )
                        nc.tensor.matmul(
                            o0[0:65, :],
                            lhsT=v_sb[:, kc, 2 * p, :],
                            rhs=ex[:, 0:512],
                            start=(kc == 0), stop=(kc == MQ - 1),
                        )
                        nc.tensor.matmul(
                            o1[0:65, :],
                            lhsT=v_sb[:, kc, 2 * p + 1, :],
                            rhs=ex[:, 512:1024],
                            start=(kc == 0), stop=(kc == MQ - 1),
                        )
                        for _ in range(per_iter):
                            if filler_queue:
                                filler_queue.popleft()()
                    for hl, o in ((0, o0), (1, o1)):
                        zs = work.tile([128, 512], DT_F32, tag="zi", name="zs_t")
                        nc.vector.tensor_copy(zs[64:65, :], o[64:65, :])
                        zb = ps_acc.tile([64, 512], DT_F32, tag="oacc", name="zb_t")
                        nc.tensor.matmul(
                            zb,
                            lhsT=ones_hi[64:65, 0:64],
                            rhs=zs[64:65, :],
                            start=True, stop=True,
                        )
                        zbi = work.tile([64, 512], DT_F32, tag="ot", name="zbi_t")
                        nc.vector.reciprocal_approx_fast(out=zbi, in_=zb)
                        if hl == 0:
                            nc.vector.tensor_mul(
                                heads_t[p][0:64, sl_q], o[0:64, :], zbi
                            )
                        else:
                            t2 = work.tile([64, 512], DT_BF, tag="t2", name="t2_t")
                            nc.vector.tensor_mul(t2, o[0:64, :], zbi)
                            nc.sync.dma_start(
                                out=heads_t[p][64:128, sl_q], in_=t2
                            )
                    if after_qb is not None:
                        after_qb(qb)

            # ---- phase 3: pair-0 attention + interleaved mc1 projections ---
            from collections import deque
            fq = deque(fillers)
            attention_pair(0, fq, per_iter=2)
            while fq:
                fq.popleft()()

            # ---- phase 4: mc1 norm burst (sqrts batched) --------------------
            for key, dst, sel in (
                ("q", qt[1], selq),
                ("k", kt[1], selk_sb[:, 128:256]),
            ):
                for n4 in range(4):
                    norm_unit(dst, sel, mc1_sq[key, n4]["sq"], n4)

            # ---- phase 5: pair-1 attention + interleaved out-projection ----
            oq = deque()

            def outproj_unit_ops(m, n2):
                sl_n = slice(n2 * 512, (n2 + 1) * 512)
                st8 = {}

                def mk_mm(c):
                    def go():
                        if c == 0:
                            st8["op"] = ps_aux.tile([128, 512], DT_F32, tag="aux", name="op_aux")
                        nc.tensor.matmul(
                            st8["op"],
                            lhsT=heads_t[c][:, m * 128:(m + 1) * 128],
                            rhs=wo_sb[:, c, sl_n],
                            start=(c == 0), stop=(c == 1),
                        )
                    return go

                def drain():
                    ob = outs.tile([128, 512], DT_F32, tag="ob", name="ob_t")
                    nc.vector.tensor_copy(ob, st8["op"])
                    nc.sync.dma_start(
                        out=outp[m * 128:(m + 1) * 128, sl_n], in_=ob
                    )
                return [mk_mm(0), mk_mm(1), drain]

            def queue_outproj(qb):
                for m in range(qb * 4, (qb + 1) * 4):
                    for n2 in range(2):
                        oq.extend(outproj_unit_ops(m, n2))

            attention_pair(1, oq, per_iter=3, after_qb=queue_outproj)
            while oq:
                oq.popleft()()

    nc.compile()
    return nc


_CACHE = {}


def _get_program():
    if "nc" not in _CACHE:
        _CACHE["nc"] = build_program()
    return _CACHE["nc"]


def make_in_maps(query, key, value, in_proj_weight, in_proj_bias,
                 out_proj_weight, out_proj_bias, tau):
    query = np.asarray(query, np.float32)
    key = np.asarray(key, np.float32)
    value = np.asarray(value, np.float32)
    W = np.asarray(in_proj_weight, np.float32)
    bias = np.asarray(in_proj_bias, np.float32)
    Wo = np.asarray(out_proj_weight, np.float32)
    tau_c = np.maximum(np.asarray(tau, np.float32).reshape(H), TAU_MIN)

    # Transposed activations per batch: (E, S) bf16
    xT = {}
    for b in range(B):
        xT["q", b] = np.ascontiguousarray(query[:, b, :].T).astype(BF16)
        xT["k", b] = np.ascontiguousarray(key[:, b, :].T).astype(BF16)
        xT["v", b] = np.ascontiguousarray(value[:, b, :].T).astype(BF16)

    selq_host = np.zeros((2, 128), np.float32)
    selq_host[0, 0:64] = 1.0
    selq_host[1, 64:128] = 1.0
    in_maps = []
    for c in range(NCORES):
        b = c // 4
        h0 = HPC * (c % 4)
        rows = slice(h0 * HD, (h0 + HPC) * HD)
        rows_k = slice(E + h0 * HD, E + (h0 + HPC) * HD)
        rows_v = slice(2 * E + h0 * HD, 2 * E + (h0 + HPC) * HD)
        # per-pair selector with 1/tau folded in for the k side
        selk = np.zeros((2, 256), np.float32)
        for mc in range(NPAIR):
            selk[0, mc * 128:mc * 128 + 64] = tau_c[h0 + 2 * mc]
            selk[1, mc * 128 + 64:(mc + 1) * 128] = tau_c[h0 + 2 * mc + 1]
        in_maps.append({
            "xq_t": xT["q", b],
            "xk_t": xT["k", b],
            "xv_t": xT["v", b],
            "wq_t": np.ascontiguousarray(W[rows, :].T).astype(BF16),
            "wk_t": np.ascontiguousarray(W[rows_k, :].T).astype(BF16),
            "wv_t": np.ascontiguousarray(W[rows_v, :].T).astype(BF16),
            "b_q": bias[rows].reshape(1, 256).astype(BF16),
            "b_k": bias[rows_k].reshape(1, 256).astype(BF16),
            "b_v": bias[rows_v].reshape(1, 256).astype(BF16),
            "wo_t": np.ascontiguousarray(Wo[:, rows].T).astype(BF16),
            "selk": selk,
            "selq": selq_host,
        })
    return in_maps


def assemble_out(results, out_proj_bias):
    bo = np.asarray(out_proj_bias, np.float32)
    out = np.zeros((S, B, E), np.float32)
    for c in range(NCORES):
        out[:, c // 4, :] += results[c]["out_p"]
    out += bo[None, None, :]
    return out


def kernel(query, key, value, in_proj_weight, in_proj_bias,
           out_proj_weight, out_proj_bias, tau):
    nc = _get_program()
    in_maps = make_in_maps(query, key, value, in_proj_weight, in_proj_bias,
                           out_proj_weight, out_proj_bias, tau)
    res = run_bass_kernel_spmd(nc, in_maps, core_ids=list(range(NCORES)))
    return assemble_out(res.results, out_proj_bias)


if __name__ == "__main__":
    import reference

    inputs = {k: np.asarray(v) for k, v in reference.setup_inputs().items()}
    out = kernel(**inputs)
    print("out shape", out.shape, out.dtype)


# revision 9
# speedup vs baseline: 1.0927x; 1.0927x over previous
"""Cosine multihead attention on 8 Trainium2 NeuronCores.

Sharding: batch*heads across cores. Core c handles batch b = c // 4 and the
4 heads [4*(c%4), 4*(c%4)+4). Each core computes its heads' q/k/v projections
(tensor-parallel slices of in_proj), full attention for its (B,H) slice, and a
partial out-projection (rank-256 contribution). The host sums the 4 partials
per batch and adds out_proj_bias.

Device layout notes:
- q,k are projected directly in transposed orientation (head_dim on
  partitions, seq on free) so QK^T needs no on-chip transpose; v is projected
  in natural orientation so it is directly the PV stationary operand.
- QK^T runs 2 heads concurrently via PE row tiling (K=64 at partition bases
  0 and 64).
- Softmax denominators come free from a ones-column appended to v (M=65 PV).
- All bf16 matmuls with fp32 PSUM accumulation; softmax/normalization math in
  fp32.
"""

import sys

if "/opt/trn_rl_repo" not in sys.path:
    sys.path.insert(0, "/opt/trn_rl_repo")

import numpy as np
import ml_dtypes

import concourse.bass as bass
import concourse.tile as tile
from concourse import bacc, mybir
from concourse.bass_utils import run_bass_kernel_spmd

S, B, E, H = 2048, 2, 1024, 16
HD = E // H            # 64
HPC = 4                # heads per core
NCORES = 8
TAU_MIN = 0.01

BF16 = ml_dtypes.bfloat16
DT_BF = mybir.dt.bfloat16
DT_F32 = mybir.dt.float32

KC_E = E // 128        # 8 contraction chunks for projections
MQ = S // 128          # 16 seq chunks of 128
NPAIR = HPC // 2       # 2 head pairs per core


def build_program():
    """Build the SPMD per-core Bass program. Same program on all 8 cores;
    all per-core differences live in the input data.

    Emission order is the schedule: mc0 projections + norms, v projection,
    then pair-0 attention with mc1 projection matmuls interleaved as PE
    fillers (keeps the PE dense so HAM stays warm), an mc1 norm burst (all
    ACT sqrts batched to avoid table thrash with Exp), then pair-1 attention
    with out-projection units interleaved, draining the tail."""
    nc = bacc.Bacc(None)

    xq = nc.dram_tensor("xq_t", [E, S], DT_BF, kind="ExternalInput")
    xk = nc.dram_tensor("xk_t", [E, S], DT_BF, kind="ExternalInput")
    xv = nc.dram_tensor("xv_t", [E, S], DT_BF, kind="ExternalInput")
    wq = nc.dram_tensor("wq_t", [E, 256], DT_BF, kind="ExternalInput")
    wk = nc.dram_tensor("wk_t", [E, 256], DT_BF, kind="ExternalInput")
    wv = nc.dram_tensor("wv_t", [E, 256], DT_BF, kind="ExternalInput")
    bq = nc.dram_tensor("b_q", [1, 256], DT_BF, kind="ExternalInput")
    bk = nc.dram_tensor("b_k", [1, 256], DT_BF, kind="ExternalInput")
    bv = nc.dram_tensor("b_v", [1, 256], DT_BF, kind="ExternalInput")
    wo = nc.dram_tensor("wo_t", [256, E], DT_BF, kind="ExternalInput")
    selk_in = nc.dram_tensor("selk", [2, 256], DT_F32, kind="ExternalInput")
    selq_in = nc.dram_tensor("selq", [2, 128], DT_F32, kind="ExternalInput")
    outp = nc.dram_tensor("out_p", [S, E], DT_F32, kind="ExternalOutput")

    with tile.TileContext(nc) as tc:
        with (
            tc.tile_pool(name="consts", bufs=1) as consts,
            tc.tile_pool(name="xin", bufs=1) as xin,
            tc.tile_pool(name="wts", bufs=1) as wts,
            tc.tile_pool(name="qk", bufs=1) as qkpool,
            tc.tile_pool(name="vsb", bufs=1) as vpool,
            tc.tile_pool(name="work", bufs=2) as work,
            tc.tile_pool(name="sqp", bufs=8) as sqp,
            tc.tile_pool(name="outs", bufs=3) as outs,
            tc.tile_pool(name="ps_mm", bufs=2, space="PSUM") as ps_mm,
            tc.tile_pool(name="ps_acc", bufs=3, space="PSUM") as ps_acc,
            tc.tile_pool(name="ps_aux", bufs=1, space="PSUM") as ps_aux,
        ):
            # ---- constants -------------------------------------------------
            ones_row = consts.tile([1, 512], DT_BF, tag="ones_row")
            nc.vector.memset(ones_row, 1.0)
            ones_hi = consts.tile([128, 64], DT_F32, tag="ones_hi")
            nc.vector.memset(ones_hi, 1.0)
            selq = consts.tile([2, 128], DT_F32, tag="selq")
            nc.sync.dma_start(out=selq, in_=selq_in[:, :])
            hsel = consts.tile([128, 2], DT_BF, tag="hsel")
            nc.vector.memset(hsel, 0.0)
            nc.vector.memset(hsel[0:64, 0:1], 1.0)
            nc.vector.memset(hsel[64:128, 1:2], 1.0)
            selk_sb = consts.tile([2, 256], DT_F32, tag="selk")
            nc.sync.dma_start(out=selk_sb, in_=selk_in[:, :])

            bq_sb = consts.tile([1, 256], DT_BF, tag="bq")
            bk_sb = consts.tile([1, 256], DT_BF, tag="bk")
            bv_sb = consts.tile([1, 256], DT_BF, tag="bv")
            nc.sync.dma_start(out=bq_sb, in_=bq[:, :])
            nc.sync.dma_start(out=bk_sb, in_=bk[:, :])
            nc.sync.dma_start(out=bv_sb, in_=bv[:, :])

            # ---- weights ---------------------------------------------------
            wq_sb = wts.tile([128, KC_E, 256], DT_BF, tag="wq")
            wk_sb = wts.tile([128, KC_E, 256], DT_BF, tag="wk")
            wv_sb = wts.tile([128, KC_E, 256], DT_BF, tag="wv")
            for c in range(KC_E):
                nc.sync.dma_start(out=wq_sb[:, c, :], in_=wq[c * 128:(c + 1) * 128, :])
                nc.gpsimd.dma_start(out=wk_sb[:, c, :], in_=wk[c * 128:(c + 1) * 128, :])
                nc.sync.dma_start(out=wv_sb[:, c, :], in_=wv[c * 128:(c + 1) * 128, :])
            wo_sb = wts.tile([128, 2, E], DT_BF, tag="wo")
            for c in range(2):
                nc.sync.dma_start(out=wo_sb[:, c, :], in_=wo[c * 128:(c + 1) * 128, :])

            # ---- activations (fully resident; xq/xv on HWDGE, xk on SWDGE
            # so the two streams run on different queues) --------------------
            xq_sb = xin.tile([128, KC_E, S], DT_BF, tag="xq")
            xk_sb = xin.tile([128, KC_E, S], DT_BF, tag="xk")
            xv_sb = xin.tile([128, KC_E, S], DT_BF, tag="xv")
            for c in range(KC_E):
                nc.sync.dma_start(out=xq_sb[:, c, :], in_=xq[c * 128:(c + 1) * 128, :])
                nc.gpsimd.dma_start(out=xk_sb[:, c, :], in_=xk[c * 128:(c + 1) * 128, :])
            for c in range(KC_E):
                nc.sync.dma_start(out=xv_sb[:, c, :], in_=xv[c * 128:(c + 1) * 128, :])

            qt = [qkpool.tile([128, S], DT_BF, tag=f"qt{p}", name=f"qt{p}")
                  for p in range(NPAIR)]
            kt = [qkpool.tile([128, S], DT_BF, tag=f"kt{p}", name=f"kt{p}")
                  for p in range(NPAIR)]
            heads_t = [qkpool.tile([128, S], DT_BF, tag=f"ht{p}", name=f"ht{p}")
                       for p in range(NPAIR)]

            def proj_unit_ops(dst, w_sb, b_sb, x_sb, mc, n4):
                """Closures for one 512-wide projection unit (8 accum matmuls
                + bias matmul + psum drain/square). Returns (ops, sq_tile_box)."""
                sl = slice(n4 * 512, (n4 + 1) * 512)
                st8 = {}
                ops = []

                def mk_mm(c):
                    def go():
                        if c == 0:
                            st8["pp"] = ps_mm.tile([128, 512], DT_F32, tag="sc", name="pp_t")
                        nc.tensor.matmul(
                            st8["pp"],
                            lhsT=w_sb[:, c, mc * 128:(mc + 1) * 128],
                            rhs=x_sb[:, c, sl],
                            start=(c == 0),
                            stop=False,
                        )
                    return go

                for c in range(KC_E):
                    ops.append(mk_mm(c))

                def bias_mm():
                    nc.tensor.matmul(
                        st8["pp"],
                        lhsT=b_sb[0:1, mc * 128:(mc + 1) * 128],
                        rhs=ones_row[0:1, 0:512],
                        start=False,
                        stop=True,
                    )
                ops.append(bias_mm)

                def drain():
                    nc.vector.tensor_copy(out=dst[:, sl], in_=st8["pp"])
                    sq = sqp.tile([128, 512], DT_BF, tag="sq", name="sq_t")
                    nc.vector.tensor_mul(sq, dst[:, sl], dst[:, sl])
                    st8["sq"] = sq
                ops.append(drain)
                return ops, st8

            def norm_unit(dst, sel, sq, n4):
                """sumsq -> sqrt -> selector-broadcast (tau folded) ->
                fast reciprocal -> in-place normalize of a 512 block."""
                sl = slice(n4 * 512, (n4 + 1) * 512)
                ss = ps_acc.tile([2, 512], DT_F32, tag="oacc", name="ss_t")
                nc.tensor.matmul(ss, lhsT=hsel, rhs=sq, start=True, stop=True)
                st = work.tile([2, 512], DT_F32, tag="st", name="st_t")
                nc.scalar.activation(st, ss, mybir.ActivationFunctionType.Sqrt)
                rb = ps_aux.tile([128, 512], DT_F32, tag="aux", name="rb_t")
                nc.tensor.matmul(rb, lhsT=sel, rhs=st, start=True, stop=True)
                rbi = work.tile([128, 512], DT_F32, tag="rbi", name="rbi_t")
                nc.vector.reciprocal_approx_fast(out=rbi, in_=rb)
                nc.vector.tensor_mul(dst[:, sl], dst[:, sl], rbi)

            # ---- phase 1: mc0 projections + norms --------------------------
            for dst, w_sb, b_sb, x_sb, sel in (
                (qt[0], wq_sb, bq_sb, xq_sb, selq),
                (kt[0], wk_sb, bk_sb, xk_sb, selk_sb[:, 0:128]),
            ):
                for n4 in range(4):
                    ops, st8 = proj_unit_ops(dst, w_sb, b_sb, x_sb, 0, n4)
                    for op in ops:
                        op()
                    norm_unit(dst, sel, st8["sq"], n4)

            # ---- phase 2: v projection (natural orientation) ---------------
            v_sb = vpool.tile([128, MQ, HPC, HD + 1], DT_BF, tag="v")
            nc.vector.memset(v_sb[:, :, :, HD:HD + 1], 1.0)
            for m in range(MQ):
                vp = ps_acc.tile([128, 256], DT_F32, tag="oacc", name="vp_t")
                for c in range(KC_E):
                    nc.tensor.matmul(
                        vp,
                        lhsT=xv_sb[:, c, m * 128:(m + 1) * 128],
                        rhs=wv_sb[:, c, :],
                        start=(c == 0),
                        stop=False,
                    )
                nc.tensor.matmul(
                    vp,
                    lhsT=ones_row[0:1, 0:128],
                    rhs=bv_sb[0:1, :],
                    start=False,
                    stop=True,
                )
                nc.vector.tensor_copy(
                    out=v_sb[:, m, :, 0:HD],
                    in_=vp.rearrange("p (h d) -> p h d", h=HPC),
                )

            from collections import deque

            def attention_pair(p, filler_queue, per_iter, after_qb=None):
                for qb in range(4):
                    sl_q = slice(qb * 512, (qb + 1) * 512)
                    o0 = ps_acc.tile([128, 512], DT_F32, tag="oacc", name="o0_t")
                    o1 = ps_acc.tile([128, 512], DT_F32, tag="oacc", name="o1_t")
                    for kc in range(MQ):
                        sc = ps_mm.tile([128, 1024], DT_F32, tag="sc", name="sc_t")
                        nc.tensor.matmul(
                            sc[:, 0:512],
                            lhsT=kt[p][0:64, kc * 128:(kc + 1) * 128],
                            rhs=qt[p][0:64, sl_q],
                            start=True, stop=True,
                        )
                        nc.tensor.matmul(
                            sc[:, 512:1024],
                            lhsT=kt[p][64:128, kc * 128:(kc + 1) * 128],
                            rhs=qt[p][64:128, sl_q],
                            start=True, stop=True,
                        )
                        ex = work.tile([128, 1024], DT_BF, tag="exp", name="ex_t")
                        nc.scalar.activation(
                            ex, sc, mybir.ActivationFunctionType.Exp
                        )
                        nc.tensor.matmul(
                            o0[0:65, :],
                            lhsT=v_sb[:, kc, 2 * p, :],
                            rhs=ex[:, 0:512],
                            start=(kc == 0), stop=(kc == MQ - 1),
                        )
                        nc.tensor.matmul(
                            o1[0:65, :],
                            lhsT=v_sb[:, kc, 2 * p + 1, :],
                            rhs=ex[:, 512:1024],
                            start=(kc == 0), stop=(kc == MQ - 1),
                        )
                        for _ in range(per_iter):
                            if filler_queue:
                                filler_queue.popleft()()
                    for hl, o in ((0, o0), (1, o1)):
                        zs = work.tile([128, 512], DT_F32, tag="zi", name="zs_t")
                        nc.vector.tensor_copy(zs[64:65, :], o[64:65, :])
                        zb = ps_aux.tile([64, 512], DT_F32, tag="aux", name="zb_t")
                        nc.tensor.matmul(
                            zb,
                            lhsT=ones_hi[64:65, 0:64],
                            rhs=zs[64:65, :],
                            start=True, stop=True,
                        )
                        zbi = work.tile([64, 512], DT_F32, tag="ot", name="zbi_t")
                        nc.vector.reciprocal_approx_fast(out=zbi, in_=zb)
                        if hl == 0:
                            nc.vector.tensor_mul(
                                heads_t[p][0:64, sl_q], o[0:64, :], zbi
                            )
                        else:
                            t2 = work.tile([64, 512], DT_BF, tag="t2", name="t2_t")
                            nc.vector.tensor_mul(t2, o[0:64, :], zbi)
                            nc.sync.dma_start(
                                out=heads_t[p][64:128, sl_q], in_=t2
                            )
                    if after_qb is not None:
                        after_qb(qb)

            def outproj_unit_ops(m, n2):
                sl_n = slice(n2 * 512, (n2 + 1) * 512)
                st8 = {}

                def mk_mm(c):
                    def go():
                        if c == 0:
                            st8["op"] = ps_mm.tile([128, 512], DT_F32, tag="sc", name="op_t")
                        nc.tensor.matmul(
                            st8["op"],
                            lhsT=heads_t[c][:, m * 128:(m + 1) * 128],
                            rhs=wo_sb[:, c, sl_n],
                            start=(c == 0), stop=(c == 1),
                        )
                    return go

                def drain():
                    ob = outs.tile([128, 512], DT_F32, tag="ob", name="ob_t")
                    nc.vector.tensor_copy(ob, st8["op"])
                    nc.sync.dma_start(
                        out=outp[m * 128:(m + 1) * 128, sl_n], in_=ob
                    )
                return [mk_mm(0), mk_mm(1), drain]

            # ---- phase 3: mc1 projections + norms (sequential) ---------
            for dst, w_sb, b_sb, x_sb, sel in (
                (qt[1], wq_sb, bq_sb, xq_sb, selq),
                (kt[1], wk_sb, bk_sb, xk_sb, selk_sb[:, 128:256]),
            ):
                for n4 in range(4):
                    ops, st8 = proj_unit_ops(dst, w_sb, b_sb, x_sb, 1, n4)
                    for op in ops:
                        op()
                    norm_unit(dst, sel, st8["sq"], n4)

            # ---- phase 4: attention (both pairs) -----------------------
            attention_pair(0, deque(), per_iter=0)
            attention_pair(1, deque(), per_iter=0)

            # ---- phase 5: out-projection -------------------------------
            for m in range(MQ):
                for n2 in range(2):
                    for op in outproj_unit_ops(m, n2):
                        op()

    nc.compile()
    return nc


_CACHE = {}


def _get_program():
    if "nc" not in _CACHE:
        _CACHE["nc"] = build_program()
    return _CACHE["nc"]


def make_in_maps(query, key, value, in_proj_weight, in_proj_bias,
                 out_proj_weight, out_proj_bias, tau):
    query = np.asarray(query, np.float32)
    key = np.asarray(key, np.float32)
    value = np.asarray(value, np.float32)
    W = np.asarray(in_proj_weight, np.float32)
    bias = np.asarray(in_proj_bias, np.float32)
    Wo = np.asarray(out_proj_weight, np.float32)
    tau_c = np.maximum(np.asarray(tau, np.float32).reshape(H), TAU_MIN)

    # Transposed activations per batch: (E, S) bf16
    xT = {}
    for b in range(B):
        xT["q", b] = np.ascontiguousarray(query[:, b, :].T).astype(BF16)
        xT["k", b] = np.ascontiguousarray(key[:, b, :].T).astype(BF16)
        xT["v", b] = np.ascontiguousarray(value[:, b, :].T).astype(BF16)

    selq_host = np.zeros((2, 128), np.float32)
    selq_host[0, 0:64] = 1.0
    selq_host[1, 64:128] = 1.0
    in_maps = []
    for c in range(NCORES):
        b = c // 4
        h0 = HPC * (c % 4)
        rows = slice(h0 * HD, (h0 + HPC) * HD)
        rows_k = slice(E + h0 * HD, E + (h0 + HPC) * HD)
        rows_v = slice(2 * E + h0 * HD, 2 * E + (h0 + HPC) * HD)
        # per-pair selector with 1/tau folded in for the k side
        selk = np.zeros((2, 256), np.float32)
        for mc in range(NPAIR):
            selk[0, mc * 128:mc * 128 + 64] = tau_c[h0 + 2 * mc]
            selk[1, mc * 128 + 64:(mc + 1) * 128] = tau_c[h0 + 2 * mc + 1]
        in_maps.append({
            "xq_t": xT["q", b],
            "xk_t": xT["k", b],
            "xv_t": xT["v", b],
            "wq_t": np.ascontiguousarray(W[rows, :].T).astype(BF16),
            "wk_t": np.ascontiguousarray(W[rows_k, :].T).astype(BF16),
            "wv_t": np.ascontiguousarray(W[rows_v, :].T).astype(BF16),
            "b_q": bias[rows].reshape(1, 256).astype(BF16),
            "b_k": bias[rows_k].reshape(1, 256).astype(BF16),
            "b_v": bias[rows_v].reshape(1, 256).astype(BF16),
            "wo_t": np.ascontiguousarray(Wo[:, rows].T).astype(BF16),
            "selk": selk,
            "selq": selq_host,
        })
    return in_maps


def assemble_out(results, out_proj_bias):
    bo = np.asarray(out_proj_bias, np.float32)
    out = np.zeros((S, B, E), np.float32)
    for c in range(NCORES):
        out[:, c // 4, :] += results[c]["out_p"]
    out += bo[None, None, :]
    return out


def kernel(query, key, value, in_proj_weight, in_proj_bias,
           out_proj_weight, out_proj_bias, tau):
    nc = _get_program()
    in_maps = make_in_maps(query, key, value, in_proj_weight, in_proj_bias,
                           out_proj_weight, out_proj_bias, tau)
    res = run_bass_kernel_spmd(nc, in_maps, core_ids=list(range(NCORES)))
    return assemble_out(res.results, out_proj_bias)


if __name__ == "__main__":
    import reference

    inputs = {k: np.asarray(v) for k, v in reference.setup_inputs().items()}
    out = kernel(**inputs)
    print("out shape", out.shape, out.dtype)
